# revision 20
# baseline (speedup 1.0000x reference)
"""BottomPool (cummax along H) for (16, 256, 128, 128) f32 on 8 TRN2 NeuronCores.

The device pipeline runs in bf16: max() selects values and introduces no
arithmetic rounding, so the only error is the host-side f32->bf16 input
cast (<= 2^-9 relative, ~10x under the 2e-2 gate) while HBM traffic —
the binding roofline — halves to 16+16 MiB per core.

Sharding: data-parallel over batch — 2 batches per core. Each core's
shard is viewed as [512 slabs, H*W] where a slab is one (b, c) image.
Partition dim = slab (128 slabs per SBUF tile).

Kernel (build_nc): h-major two-level blocked scan (step1 in-block /
step2 block chain / step3 prefix distribute) on the DVE, ~1.9 packed
bf16 passes (~73us/core busy) overlapped with full-shard SBUF prefetch.
Loads ride the SP HWDGE ring, stores the ACT HWDGE ring (SWDGE retired:
the loser-core straggler engine is NOT descriptor-ring contention — it
is HBM-pair arbitration; all-HWDGE is equal-or-better and simpler).
x/out DRAM rows are pitch-padded + column-offset (XPAD/XOFF/OPAD/OOFF)
to shift per-engine HBM channel phase: measured to clear the DMA_15-
type straggler on cores 0/6 (max-core unchanged ~108us, mean -3us).
(build_nc_scan, kept for reference, mapped the cummax onto the
``tensor_tensor_scan`` ISA op in a host-transposed w-major layout — one
DVE op per chunk — but on hardware the scan measured 2.1 cyc/elem vs
the 0.5 of packed TT ops AND miscomputed, so it is dead code.)

walrus codegen accepts only a small number of sync waits per instruction
(one for DMA pseudo-instructions), but Tile's sem assigner is not
transitively minimal and can attach more. strip_implied_waits() removes
waits that are provably implied: a wait is redundant when the kept waits'
completion closure (instructions that must have completed, including
same-HWDGE-ring FIFO predecessors of completed DMAs) already forces the
waited semaphore to the required value. Two structural invariants keep
every DMA at one wait: (a) bufs < n_tiles, so slot-reuse WAR chains let
the stripper witness store completion, and (b) every store gets a 1-elem
DVE "witness" copy a couple of chunks later that folds its DMAHW
lane-sem completion into the DVE tick stream.
"""

import ml_dtypes
import numpy as np

from concourse import bass, mybir, tile
from concourse.bass_utils import run_bass_kernel_spmd

N_CORES = 8
BATCH, CH, H, W = 16, 256, 128, 128
FREE = H * W
P = 128                      # slabs per tile = SBUF partitions
SLABS = (BATCH // N_CORES) * CH  # 512 slabs per core
DT = mybir.dt.bfloat16
NP_DT = ml_dtypes.bfloat16
XPAD = 96                     # DRAM row-pitch pad (elems) for x, see build_nc
XOFF = 32                     # data column offset inside the padded row
OPAD = 224                    # same for out
OOFF = 96

_NC_CACHE = {}


def _strip_instruction_waits(nc, max_waits={"InstDMACopy": 1, "InstDrain": 1}):
    insts = []
    for f in nc.m.functions:
        for b in f.blocks:
            insts.extend(b.instructions)

    # Monotone-sem updater table: sem_id -> [(cum_value_after, inst_idx)].
    # Sems touched by non-monotone updates are excluded entirely.
    poisoned = set()
    cum = {}
    updaters = {}
    inst_updates = [[] for _ in insts]  # idx -> [(sem_id, cum_after)]
    for idx, ins in enumerate(insts):
        si = ins.sync_info
        if si is None:
            continue
        for u in si.on_update:
            if u.update_mode == "sem-add-imm" and u.update_reg is None:
                val = u.update_value
            elif u.update_mode == "sem-inc":
                val = 1
            else:
                poisoned.add(u.id)
                continue
            cum[u.id] = cum.get(u.id, 0) + val
            updaters.setdefault(u.id, []).append((cum[u.id], idx))
            inst_updates[idx].append((u.id, cum[u.id]))

    # Same-HWDGE-ring FIFO order: DMAs issued on one ring complete in
    # program order, so a later DMA's completion implies all earlier ones.
    ring_pos = {}   # inst_idx -> (queue, position)
    ring_members = {}  # queue -> [inst_idx in order]
    for idx, ins in enumerate(insts):
        if isinstance(ins, mybir.InstDMACopy):
            q = ins.queue
            ring_members.setdefault(q, []).append(idx)
            ring_pos[idx] = (q, len(ring_members[q]) - 1)

    inst_waits = []
    for ins in insts:
        si = ins.sync_info
        ws = []
        if si is not None:
            for w in si.on_wait:
                if w.wait_mode == "sem-ge-imm" and w.wait_reg is None:
                    ws.append((w.id, w.wait_value, True))
                else:
                    ws.append((w.id, w.wait_value, False))
        inst_waits.append(ws)

    def facts_from(seed_waits):
        """Fixpoint: semaphore lower bounds guaranteed once seed_waits hold."""
        facts = {}
        for sid, v, clean in seed_waits:
            if clean and sid not in poisoned:
                facts[sid] = max(facts.get(sid, 0), v)
        completed = set()
        changed = True
        while changed:
            changed = False
            for sid, v in list(facts.items()):
                for cval, idx in updaters.get(sid, []):
                    if cval > v:
                        break
                    if idx not in completed:
                        completed.add(idx)
                        changed = True
            for idx in list(completed):
                rp = ring_pos.get(idx)
                if rp is not None:
                    q, pos = rp
                    for pidx in ring_members[q][:pos]:
                        if pidx not in completed:
                            completed.add(pidx)
                            changed = True
            for idx in list(completed):
                for sid, v, clean in inst_waits[idx]:
                    if clean and sid not in poisoned and facts.get(sid, 0) < v:
                        facts[sid] = v
                        changed = True
                for sid, cval in inst_updates[idx]:
                    if sid not in poisoned and facts.get(sid, 0) < cval:
                        facts[sid] = cval
                        changed = True
        return facts

    # Engine queues issue strictly in program order, so by the time an
    # instruction issues, every wait of every EARLIER instruction on its
    # engine queue has been satisfied — those waits are free facts for the
    # implication closure (the Tile sem assigner itself relies on exactly
    # this order when it omits duplicate same-queue waits).
    prior_waits = [[] for _ in insts]
    eng_acc = {}
    for idx, ins in enumerate(insts):
        eng = ins.engine
        acc = eng_acc.setdefault(eng, [])
        prior_waits[idx] = list(acc)
        acc.extend(inst_waits[idx])

    n_stripped = 0
    for idx, ins in enumerate(insts):
        si = ins.sync_info
        if si is None or len(si.on_wait) <= 1:
            continue
        kept = list(si.on_wait)

        def key(w):
            return (w.id, w.wait_value, w.wait_mode == "sem-ge-imm" and w.wait_reg is None)

        progress = True
        while len(kept) >= 1 and progress:
            progress = False
            for w in list(kept):
                sid, v, clean = key(w)
                if not clean or sid in poisoned:
                    continue
                others = [key(k) for k in kept if k is not w] + prior_waits[idx]
                if facts_from(others).get(sid, 0) >= v:
                    kept.remove(w)
                    n_stripped += 1
                    progress = True
                    break
        limit = max_waits.get(type(ins).__name__)
        if limit is not None and len(kept) > limit:
            raise RuntimeError(
                f"{type(ins).__name__} {ins.name} still has {len(kept)} waits: "
                f"{[(w.ant_name, w.wait_value) for w in kept]}"
            )
        if len(kept) != len(si.on_wait):
            ins.sync_info = mybir.SyncInfo(on_wait=kept, on_update=list(si.on_update))

    # Second sweep: drop vacuous same-engine waits on the DVE. The DVE
    # retires strictly in order (per-op DRAIN), so a wait on the DVE's own
    # completion sem whose target value is reached by an earlier DVE
    # instruction in the stream is satisfied by construction.
    dve = mybir.EngineType.DVE
    stream_pos = {}
    pos = 0
    for idx, ins in enumerate(insts):
        if ins.engine == dve:
            stream_pos[idx] = pos
            pos += 1
    upd_engine_ok = {}  # sem_id -> True if all updaters are DVE non-DMA instrs
    for sid, ups in updaters.items():
        upd_engine_ok[sid] = all(
            insts[i].engine == dve
            and not isinstance(insts[i], (mybir.InstDMACopy, mybir.InstCollectiveCompute))
            for _, i in ups
        )
    for idx, ins in enumerate(insts):
        if ins.engine != dve:
            continue
        si = ins.sync_info
        if si is None or not si.on_wait:
            continue
        kept = []
        for w in si.on_wait:
            if (
                w.wait_mode == "sem-ge-imm"
                and w.wait_reg is None
                and w.id not in poisoned
                and upd_engine_ok.get(w.id)
            ):
                ups = updaters.get(w.id, [])
                first = next((i for cv, i in ups if cv >= w.wait_value), None)
                if first is not None and stream_pos.get(first, 1 << 60) < stream_pos[idx]:
                    n_stripped += 1
                    continue
            kept.append(w)
        if len(kept) != len(si.on_wait):
            ins.sync_info = mybir.SyncInfo(on_wait=kept, on_update=list(si.on_update))
    return n_stripped


def build_nc_scan(n_slabs: int = SLABS, bufs: int = 3, chunks: int = 2,
                  first_splits: int = 2, tail_groups: int = 2,
                  witness_lag: int = 2, strip: bool = True):
    """W-major hardware-scan kernel. Each slab arrives transposed to
    [w, h] (h contiguous), so one tensor_tensor_scan per chunk computes
    the per-column cummax: the mask input (-3e38 at h==0, else 0) resets
    the fp32 scan state at every column start, which also makes chunks
    and tiles fully independent — no spare columns, no cross-chunk state.
    Loads ride the SP HWDGE ring, stores the ACT ring. Stores need their
    lane-reuse waits strippable: each store gets a 1-elem DVE witness
    copy `witness_lag` chunks later (see module docstring)."""
    CHF = FREE // chunks
    assert CHF % H == 0 and n_slabs % P == 0
    n_tiles = n_slabs // P
    assert bufs < n_tiles, "slot-reuse WAR chains require bufs < n_tiles"

    nc = bass.Bass("TRN2", target_bir_lowering=False, debug=False)
    x = nc.dram_tensor("x", [n_slabs, FREE], DT, kind="ExternalInput").ap()
    out = nc.dram_tensor("out", [n_slabs, FREE], DT, kind="ExternalOutput").ap()

    pending = []               # stored chunk APs awaiting a witness copy
    with tile.TileContext(nc) as tc:
        with tc.tile_pool(name="mask", bufs=1) as mpool, \
                tc.tile_pool(name="work", bufs=bufs) as pool:
            mask = mpool.tile([P, CHF], DT)
            nc.vector.memset(mask, 0.0)
            mv = mask.rearrange("p (c h) -> p c h", h=H)
            nc.vector.memset(mv[:, :, 0:1], -3.0e38)
            for t in range(n_tiles):
                tl = pool.tile([P, FREE], DT)
                xrow = x[t * P:(t + 1) * P, :]
                orow = out[t * P:(t + 1) * P, :]
                for c in range(chunks):
                    c0, c1 = c * CHF, (c + 1) * CHF
                    ch = tl[:, c0:c1]
                    first_chunk = t == 0 and c == 0
                    last_chunk = t == n_tiles - 1 and c == chunks - 1
                    if last_chunk:
                        # all outstanding stores must be witnessed before
                        # the tail stores issue (their lane-reuse preds)
                        for pch in pending:
                            nc.vector.tensor_copy(
                                pch[0:1, CHF - 1:CHF], pch[0:1, CHF - 1:CHF])
                        pending.clear()
                    # pieces: split the first chunk so the DVE starts on
                    # piece 0 while piece 1 loads; split the tail chunk so
                    # the final store is small
                    pieces = (first_splits if first_chunk
                              else tail_groups if last_chunk else 1)
                    pp = CHF // pieces
                    assert pp % H == 0
                    for pc in range(pieces):
                        s0, s1 = pc * pp, (pc + 1) * pp
                        nc.sync.dma_start(
                            ch[:, s0:s1], xrow[:, c0 + s0:c0 + s1])
                        nc.vector.tensor_tensor_scan(
                            ch[:, s0:s1], mask[:, 0:pp], ch[:, s0:s1],
                            0.0, mybir.AluOpType.add, mybir.AluOpType.max)
                        if last_chunk:
                            nc.scalar.dma_start(
                                orow[:, c0 + s0:c0 + s1], ch[:, s0:s1])
                    if not last_chunk:
                        nc.scalar.dma_start(orow[:, c0:c1], ch)
                        pending.append(ch)
                        if len(pending) > witness_lag:
                            pch = pending.pop(0)
                            nc.vector.tensor_copy(
                                pch[0:1, CHF - 1:CHF], pch[0:1, CHF - 1:CHF])
                    else:
                        # joiner: fold the final store's completion into the
                        # DVE stream so the kernel drain needs one ring wait
                        nc.vector.tensor_copy(
                            ch[0:1, CHF - 1:CHF], ch[0:1, CHF - 1:CHF])

    if strip:
        _strip_instruction_waits(nc)
    return nc


def build_nc(n_slabs: int = SLABS, bufs: int = 4, blocks: int = 16, halves: int = 1,
             first_splits: int = 2, dma_splits: int = 1, witness_lag: int = 4,
             load_splits: int = 1, xpad: int = 0, opad: int = 0,
             strip: bool = True):
    """h-major two-level blocked scan, bf16, pure DVE compute.

    halves: split each tile's load/compute/store into this many h-chunks.
    Each SBUF tile carries a W-wide "spare" column ahead of the data
    holding the previous tile's running max slice, so every block's
    prev-prefix slice sits exactly W elements before the block start —
    step2 and step3 use one uniform AP family and step3 collapses to one
    stride-0-broadcast op per chunk.

    Loads ride the SP HWDGE ring; chunk stores ride SWDGE (own DMASW sem
    lanes — loads keep all 8 DMAHW lanes to themselves). Each is issued
    as dma_splits sub-DMAs: 1 MiB transfers give the inter-core HBM
    arbiter finer interleave boundaries, which evens out the per-run
    "loser core" whose streams starve while its stack partner bursts.
    The tail stores quarter-granular entirely on the otherwise-idle ACT
    HWDGE ring — on a starved core the SWDGE ring is backlogged exactly
    then, and the tail must not queue behind it.

    bufs=4 = n_tiles holds the ENTIRE 16 MiB input shard in SBUF
    (130 KiB/partition): loads free-run at full HBM rate from t=0 with no
    WAR coupling to stores/compute, which rides out the multi-us DMA
    outages observed when this core's HBM-stack partner bursts. Without
    WAR chains the stripper cannot witness store completion, so each
    chunk store gets a 1-elem DVE "witness" copy witness_lag chunks later
    (far enough that the store has already completed — a shorter lag
    stalls the DVE, measured -6us/core at lag 2) that folds its DMASW
    lane-sem into the DVE tick stream, keeping every DMA at one wait.

    first_splits splits the first chunk's load so the DVE starts ~3us
    sooner."""
    B = blocks
    S = H // B
    assert n_slabs % P == 0
    assert B % halves == 0
    n_tiles = n_slabs // P
    BH = B // halves           # blocks per chunk
    CHF = FREE // halves       # free elems per chunk
    BW = S * W                 # elements per block

    nc = bass.Bass("TRN2", target_bir_lowering=False, debug=False)
    # xpad/opad: extra elements of DRAM row pitch — shifts each row's HBM
    # channel phase to decorrelate the per-engine address streams from the
    # stack partner's (straggler-engine mitigation experiment)
    xoff = XOFF if xpad else 0
    ooff = OOFF if opad else 0
    x = nc.dram_tensor("x", [n_slabs, FREE + xpad], DT,
                       kind="ExternalInput").ap()[:, xoff:xoff + FREE]
    out = nc.dram_tensor("out", [n_slabs, FREE + opad], DT,
                         kind="ExternalOutput").ap()[:, ooff:ooff + FREE]

    # tile layout: [spare0 | chunk0 | spare1 | chunk1 | ...] — each chunk's
    # spare (W elems) holds the running-max slice entering that chunk, so
    # every block's prev-prefix slice sits exactly W elems before the block
    TW = halves * W + FREE
    pending = []               # stored chunk APs awaiting a witness copy
    with tile.TileContext(nc) as tc:
        with tc.tile_pool(name="work", bufs=bufs) as pool:
            for t in range(n_tiles):
                tl = pool.tile([P, TW], DT)
                # each tile is an independent set of slabs; block 0 has no
                # predecessor, so its step2 link and step3 term (max with
                # the -inf spare) are numeric no-ops and are skipped —
                # which also makes the spare memset dead
                xrow = x[t * P:(t + 1) * P, :]
                orow = out[t * P:(t + 1) * P, :]
                for h in range(halves):
                    base = h * (W + CHF)
                    c0, c1 = h * CHF, (h + 1) * CHF
                    ch = tl[:, base + W:base + W + CHF]
                    v = ch.rearrange("p (b s w) -> p b s w", b=BH, s=S, w=W)
                    prevlast = tl[:, base:base + CHF].rearrange(
                        "p (b s w) -> p b s w", b=BH, s=S, w=W)[:, :, 0, :]
                    first_chunk = t == 0 and h == 0
                    last_chunk = t == n_tiles - 1 and h == halves - 1

                    if not last_chunk:
                        # loads (SP HWDGE ring) + step1 (in-block scan),
                        # piecewise for the first chunk so the DVE starts
                        # on piece 0 while piece 1 loads. Loads must NOT be
                        # spread across both HWDGE rings: the SDMA engines
                        # round-robin across queues, and 2 load queues vs 1
                        # store queue starves the store stream (measured
                        # +6us mean/core).
                        done_segments = False
                        if halves == 1 and t < 3 and first_splits == 2:
                            # tiles 0-1, fully per-segment: load/step1/
                            # step2/step3/store of each 2 MiB segment runs
                            # while the next segment's load is in flight.
                            # Early tiles have no prefetch cushion yet, so
                            # whole-tile (4 MiB) step1 granularity stalls
                            # the DVE ~10us when HBM bandwidth is tight
                            # (measured); later tiles run ahead of the DVE
                            # and keep the cheaper 7-fat-op form.
                            seg = BH // 2
                            for hh in range(2):
                                sb0, sb1 = hh * seg, (hh + 1) * seg
                                if t == 0 and hh == 0:
                                    # ladder the very first load (512K,
                                    # 512K, 1M): the DVE's first step1
                                    # starts on 512 KiB instead of 2 MiB
                                    # (~4us earlier for ~1us of extra op
                                    # overhead, paid once)
                                    ladder = [(sb0, sb0 + 2),
                                              (sb0 + 2, sb0 + 4),
                                              (sb0 + 4, sb1)]
                                elif load_splits > 1:
                                    lsp = seg // load_splits
                                    ladder = [(sb0 + i * lsp,
                                               sb0 + (i + 1) * lsp)
                                              for i in range(load_splits)]
                                else:
                                    ladder = [(sb0, sb1)]
                                for lb0, lb1 in ladder:
                                    nc.sync.dma_start(
                                        ch[:, lb0 * BW:lb1 * BW],
                                        xrow[:, c0 + lb0 * BW:c0 + lb1 * BW])
                                    for j in range(1, S):
                                        nc.vector.tensor_max(
                                            v[:, lb0:lb1, j, :],
                                            v[:, lb0:lb1, j, :],
                                            v[:, lb0:lb1, j - 1, :])
                                a0 = max(sb0, 1)
                                for b in range(a0, sb1):
                                    nc.vector.tensor_max(
                                        v[:, b, S - 1, :], v[:, b, S - 1, :],
                                        prevlast[:, b, :])
                                pb = prevlast[:, a0:sb1, :].unsqueeze(2) \
                                    .broadcast_to([P, sb1 - a0, S - 1, W])
                                nc.vector.tensor_max(
                                    v[:, a0:sb1, 0:S - 1, :],
                                    v[:, a0:sb1, 0:S - 1, :], pb)
                                ssp = seg // dma_splits
                                for sp in range(dma_splits):
                                    qb0 = sb0 + sp * ssp
                                    qb1 = sb0 + (sp + 1) * ssp
                                    nc.scalar.dma_start(
                                        orow[:, c0 + qb0 * BW:c0 + qb1 * BW],
                                        ch[:, qb0 * BW:qb1 * BW])
                                    pending.append(
                                        ch[0:1, qb1 * BW - 1:qb1 * BW])
                            done_segments = True
                        elif first_chunk:
                            pieces = first_splits
                            bpp = BH // pieces
                            for pc in range(pieces):
                                b0, b1 = pc * bpp, (pc + 1) * bpp
                                nc.sync.dma_start(
                                    ch[:, b0 * BW:b1 * BW],
                                    xrow[:, c0 + b0 * BW:c0 + b1 * BW])
                                for j in range(1, S):
                                    nc.vector.tensor_max(
                                        v[:, b0:b1, j, :], v[:, b0:b1, j, :],
                                        v[:, b0:b1, j - 1, :])
                        else:
                            sw = CHF // dma_splits
                            for sp in range(dma_splits):
                                nc.sync.dma_start(
                                    ch[:, sp * sw:(sp + 1) * sw],
                                    xrow[:, c0 + sp * sw:c0 + (sp + 1) * sw])
                            for j in range(1, S):
                                nc.vector.tensor_max(
                                    v[:, :, j, :], v[:, :, j, :],
                                    v[:, :, j - 1, :])

                        # witness stores issued witness_lag chunks ago:
                        # 1-elem WAR copies folding their DMASW lane-sems
                        # into the DVE tick stream (see docstring). With
                        # halves=1 there are at most 8 SWDGE stores — no
                        # lane reuse — so this never fires.
                        while len(pending) >= witness_lag * dma_splits:
                            pch = pending.pop(0)
                            nc.vector.tensor_copy(pch, pch)

                        if halves == 1 and done_segments:
                            pass
                        elif halves == 1:
                            # hybrid cadence: tile-granular step1 (7 fat
                            # ops) but step2 chained in two 8-block
                            # segments, each followed by its half's step3
                            # and a 2 MiB store — the store stream keeps
                            # the half-tile rhythm that a single tile-end
                            # store would destroy (measured +8.7us drain)
                            seg = BH // 2
                            for hh in range(2):
                                sb0, sb1 = hh * seg, (hh + 1) * seg
                                a0 = max(sb0, 1)
                                for b in range(a0, sb1):
                                    nc.vector.tensor_max(
                                        v[:, b, S - 1, :], v[:, b, S - 1, :],
                                        prevlast[:, b, :])
                                pb = prevlast[:, a0:sb1, :].unsqueeze(2) \
                                    .broadcast_to([P, sb1 - a0, S - 1, W])
                                nc.vector.tensor_max(
                                    v[:, a0:sb1, 0:S - 1, :],
                                    v[:, a0:sb1, 0:S - 1, :], pb)
                                nc.scalar.dma_start(
                                    orow[:, c0 + sb0 * BW:c0 + sb1 * BW],
                                    ch[:, sb0 * BW:sb1 * BW])
                                pending.append(
                                    ch[0:1, sb1 * BW - 1:sb1 * BW])
                        else:
                            # step2: chain block-last slices through the spare
                            for b in range(1, BH):
                                nc.vector.tensor_max(
                                    v[:, b, S - 1, :], v[:, b, S - 1, :],
                                    prevlast[:, b, :])
                            # bridge the running max into the next chunk's
                            # spare
                            bridge = None
                            if h + 1 < halves:
                                bridge = nc.vector.tensor_copy(
                                    tl[:, base + W + CHF:base + 2 * W + CHF],
                                    v[:, BH - 1, S - 1, :])
                            # step3: one op — prev-block prefix into slices
                            # 0..S-2
                            pb = prevlast.unsqueeze(2).broadcast_to(
                                [P, BH, S - 1, W])
                            s3 = nc.vector.tensor_max(
                                v[:, :, 0:S - 1, :], v[:, :, 0:S - 1, :], pb)
                            if bridge is not None:
                                # keep the bridge's DVE tick below the
                                # store's wait target so the slot's readers
                                # stay within it
                                tile.add_dep_helper(
                                    s3.ins, bridge.ins, sync=False,
                                    reason="bridge copy before step3 so slot "
                                           "readers stay under the store's "
                                           "DVE wait")
                            sw = CHF // dma_splits
                            for sp in range(dma_splits):
                                st = nc.scalar.dma_start(
                                    orow[:, c0 + sp * sw:c0 + (sp + 1) * sw],
                                    ch[:, sp * sw:(sp + 1) * sw])
                                # force an explicit DVE wait: Tile would
                                # give later sub-stores only pool-queue
                                # order, leaving them an unstrippable
                                # {lane-reuse, sub-load-RAW} wait pair; a
                                # DVE>=step3 wait implies both
                                tile.add_dep_helper(
                                    st.ins, s3.ins, sync=True,
                                    reason="sub-store's single DVE wait "
                                           "implies its lane-reuse and "
                                           "sub-load waits")
                                pending.append(
                                    ch[0:1, (sp + 1) * sw - 1:(sp + 1) * sw])
                    else:
                        # tail: the pipeline runs dry here, so feed the DMA
                        # to the very end — half-granular loads+step1,
                        # quarter-granular step3+stores alternating SWDGE
                        # and the ACT ring so the drain runs on two rings
                        # in parallel
                        BQ = BH // 2
                        CQ = CHF // 2
                        for half in range(2):
                            hb0 = half * BQ
                            nc.sync.dma_start(
                                ch[:, half * CQ:(half + 1) * CQ],
                                xrow[:, c0 + half * CQ:c0 + (half + 1) * CQ])
                            for j in range(1, S):
                                nc.vector.tensor_max(
                                    v[:, hb0:hb0 + BQ, j, :],
                                    v[:, hb0:hb0 + BQ, j, :],
                                    v[:, hb0:hb0 + BQ, j - 1, :])
                            for b in range(max(hb0, 1), hb0 + BQ):
                                nc.vector.tensor_max(
                                    v[:, b, S - 1, :], v[:, b, S - 1, :],
                                    prevlast[:, b, :])
                            if half == 0:
                                qbnds = [hb0, hb0 + BQ // 2, hb0 + BQ]
                            else:
                                # last half: taper the pieces so the final
                                # store (the pipeline's drain) is 512 KiB
                                qbnds = [hb0, hb0 + BQ // 2,
                                         hb0 + 3 * BQ // 4, hb0 + BQ]
                            for q in range(len(qbnds) - 1):
                                qb0, qb1 = qbnds[q], qbnds[q + 1]
                                qc0 = qb0 * S * W
                                qc1 = qb1 * S * W
                                qa = max(qb0, 1)
                                pq = prevlast[:, qa:qb1, :].unsqueeze(2) \
                                    .broadcast_to([P, qb1 - qa, S - 1, W])
                                nc.vector.tensor_max(
                                    v[:, qa:qb1, 0:S - 1, :],
                                    v[:, qa:qb1, 0:S - 1, :], pq)
                                eng = nc.scalar
                                eng.dma_start(
                                    orow[:, c0 + qc0:c0 + qc1], ch[:, qc0:qc1])
                        # joiners: 1-elem WAR copies handing the DVE the two
                        # rings' final-store completion waits, so the kernel
                        # tail drain reduces to engine waits. Stores left in
                        # `pending` stay unwitnessed: ring-FIFO closure from
                        # the final SWDGE store covers them.
                        nc.vector.tensor_copy(
                            ch[0:1, 2 * CQ // 2:2 * CQ // 2 + 1],
                            ch[0:1, 2 * CQ // 2:2 * CQ // 2 + 1])
                        nc.vector.tensor_copy(
                            ch[0:1, CHF - 1:CHF], ch[0:1, CHF - 1:CHF])

    if strip:
        # validate strippability even when returning the unstripped module
        # (CoreSim's race detector doesn't model same-engine in-order
        # retirement, so sim runs pass strip=False)
        _strip_instruction_waits(nc)
    return nc


def _get_nc():
    key = "default"
    if key not in _NC_CACHE:
        # h-major blocked kernel only: tensor_tensor_scan measured 2.1
        # cyc/elem on HW (vs 0.5 for packed bf16 TT) AND miscomputed, so
        # the scan path is dead. The Tile scheduler is not perfectly
        # deterministic across processes; if a schedule ever leaves a DMA
        # with >1 sync wait the stripper raises. Retry, then fall back to
        # coarser structures whose stripping is trivially easy.
        nc = None
        layout = "hmajor"
        for attempt in range(3):
            try:
                nc = build_nc(xpad=XPAD, opad=OPAD)
                break
            except RuntimeError:
                continue
        if nc is None:
            for kwargs in (
                dict(first_splits=1),
                dict(first_splits=1, bufs=3),
                dict(first_splits=1, halves=1, bufs=3),
            ):
                try:
                    nc = build_nc(xpad=XPAD, opad=OPAD, **kwargs)
                    break
                except RuntimeError:
                    continue
        assert nc is not None, "all kernel builds failed wait-stripping"
        _NC_CACHE[key] = (nc, layout)
    return _NC_CACHE[key]


def _shard(x: np.ndarray, layout: str):
    per = BATCH // N_CORES
    xb = x.astype(NP_DT)
    if layout == "wmajor":
        xb = xb.transpose(0, 1, 3, 2)  # [B, C, W, H] — h contiguous per col
    shards = []
    for i in range(N_CORES):
        s = np.ascontiguousarray(xb[i * per:(i + 1) * per]).reshape(SLABS, FREE)
        if XPAD:
            sp = np.zeros((SLABS, FREE + XPAD), dtype=NP_DT)
            sp[:, XOFF:XOFF + FREE] = s
            s = sp
        shards.append(s)
    return shards


def _unshard(outs, layout: str):
    per = BATCH // N_CORES
    outs = [o[:, OOFF:OOFF + FREE] if OPAD else o for o in outs]
    if layout == "wmajor":
        shards = [o.reshape(per, CH, W, H).transpose(0, 1, 3, 2) for o in outs]
    else:
        shards = [o.reshape(per, CH, H, W) for o in outs]
    return np.concatenate([s.astype(np.float32) for s in shards], axis=0)


def run(x: np.ndarray, trace: bool = False, **kwargs):
    """Run on hardware; returns (full_output, BassKernelResults)."""
    x = np.asarray(x, dtype=np.float32)
    assert x.shape == (BATCH, CH, H, W), x.shape
    nc, layout = _get_nc()
    in_maps = [{"x": s} for s in _shard(x, layout)]
    res = run_bass_kernel_spmd(
        nc, in_maps, core_ids=list(range(N_CORES)), trace=trace, **kwargs
    )
    out = _unshard([res.results[i]["out"] for i in range(N_CORES)], layout)
    return out, res


def kernel(x) -> np.ndarray:
    out, _ = run(np.asarray(x), trace=False)
    return out



# revision 21
# speedup vs baseline: 1.0379x; 1.0379x over previous
"""BottomPool (cummax along H) for (16, 256, 128, 128) f32 on 8 TRN2 NeuronCores.

The device pipeline runs in bf16: max() selects values and introduces no
arithmetic rounding, so the only error is the host-side f32->bf16 input
cast (<= 2^-9 relative, ~10x under the 2e-2 gate) while HBM traffic —
the binding roofline — halves to 16+16 MiB per core.

Sharding: data-parallel over batch — 2 batches per core. Each core's
shard is viewed as [512 slabs, H*W] where a slab is one (b, c) image.
Partition dim = slab (128 slabs per SBUF tile).

Kernel (build_nc): h-major two-level blocked scan (step1 in-block /
step2 block chain / step3 prefix distribute) on the DVE, ~1.9 packed
bf16 passes (~73us/core busy) overlapped with full-shard SBUF prefetch.
Loads ride the SP HWDGE ring, stores the ACT HWDGE ring (SWDGE retired:
the loser-core straggler engine is NOT descriptor-ring contention — it
is HBM-pair arbitration; all-HWDGE is equal-or-better and simpler).
x/out DRAM rows are pitch-padded + column-offset (XPAD/XOFF/OPAD/OOFF)
to shift per-engine HBM channel phase: measured to clear the DMA_15-
type straggler on cores 0/6 (max-core unchanged ~108us, mean -3us).
(build_nc_scan, kept for reference, mapped the cummax onto the
``tensor_tensor_scan`` ISA op in a host-transposed w-major layout — one
DVE op per chunk — but on hardware the scan measured 2.1 cyc/elem vs
the 0.5 of packed TT ops AND miscomputed, so it is dead code.)

walrus codegen accepts only a small number of sync waits per instruction
(one for DMA pseudo-instructions), but Tile's sem assigner is not
transitively minimal and can attach more. strip_implied_waits() removes
waits that are provably implied: a wait is redundant when the kept waits'
completion closure (instructions that must have completed, including
same-HWDGE-ring FIFO predecessors of completed DMAs) already forces the
waited semaphore to the required value. Two structural invariants keep
every DMA at one wait: (a) bufs < n_tiles, so slot-reuse WAR chains let
the stripper witness store completion, and (b) every store gets a 1-elem
DVE "witness" copy a couple of chunks later that folds its DMAHW
lane-sem completion into the DVE tick stream.
"""

import ml_dtypes
import numpy as np

from concourse import bass, mybir, tile
from concourse.bass_utils import run_bass_kernel_spmd

N_CORES = 8
BATCH, CH, H, W = 16, 256, 128, 128
FREE = H * W
P = 128                      # slabs per tile = SBUF partitions
SLABS = (BATCH // N_CORES) * CH  # 512 slabs per core
DT = mybir.dt.bfloat16
NP_DT = ml_dtypes.bfloat16
XPAD = 64                     # DRAM row-pitch pad (elems) for x, see build_nc
XOFF = 0                      # data column offset inside the padded row
OPAD = 64                     # same for out
OOFF = 0

_NC_CACHE = {}


def _strip_instruction_waits(nc, max_waits={"InstDMACopy": 1, "InstDrain": 1}):
    insts = []
    for f in nc.m.functions:
        for b in f.blocks:
            insts.extend(b.instructions)

    # Monotone-sem updater table: sem_id -> [(cum_value_after, inst_idx)].
    # Sems touched by non-monotone updates are excluded entirely.
    poisoned = set()
    cum = {}
    updaters = {}
    inst_updates = [[] for _ in insts]  # idx -> [(sem_id, cum_after)]
    for idx, ins in enumerate(insts):
        si = ins.sync_info
        if si is None:
            continue
        for u in si.on_update:
            if u.update_mode == "sem-add-imm" and u.update_reg is None:
                val = u.update_value
            elif u.update_mode == "sem-inc":
                val = 1
            else:
                poisoned.add(u.id)
                continue
            cum[u.id] = cum.get(u.id, 0) + val
            updaters.setdefault(u.id, []).append((cum[u.id], idx))
            inst_updates[idx].append((u.id, cum[u.id]))

    # Same-HWDGE-ring FIFO order: DMAs issued on one ring complete in
    # program order, so a later DMA's completion implies all earlier ones.
    ring_pos = {}   # inst_idx -> (queue, position)
    ring_members = {}  # queue -> [inst_idx in order]
    for idx, ins in enumerate(insts):
        if isinstance(ins, mybir.InstDMACopy):
            q = ins.queue
            ring_members.setdefault(q, []).append(idx)
            ring_pos[idx] = (q, len(ring_members[q]) - 1)

    inst_waits = []
    for ins in insts:
        si = ins.sync_info
        ws = []
        if si is not None:
            for w in si.on_wait:
                if w.wait_mode == "sem-ge-imm" and w.wait_reg is None:
                    ws.append((w.id, w.wait_value, True))
                else:
                    ws.append((w.id, w.wait_value, False))
        inst_waits.append(ws)

    def facts_from(seed_waits):
        """Fixpoint: semaphore lower bounds guaranteed once seed_waits hold."""
        facts = {}
        for sid, v, clean in seed_waits:
            if clean and sid not in poisoned:
                facts[sid] = max(facts.get(sid, 0), v)
        completed = set()
        changed = True
        while changed:
            changed = False
            for sid, v in list(facts.items()):
                for cval, idx in updaters.get(sid, []):
                    if cval > v:
                        break
                    if idx not in completed:
                        completed.add(idx)
                        changed = True
            for idx in list(completed):
                rp = ring_pos.get(idx)
                if rp is not None:
                    q, pos = rp
                    for pidx in ring_members[q][:pos]:
                        if pidx not in completed:
                            completed.add(pidx)
                            changed = True
            for idx in list(completed):
                for sid, v, clean in inst_waits[idx]:
                    if clean and sid not in poisoned and facts.get(sid, 0) < v:
                        facts[sid] = v
                        changed = True
                for sid, cval in inst_updates[idx]:
                    if sid not in poisoned and facts.get(sid, 0) < cval:
                        facts[sid] = cval
                        changed = True
        return facts

    # Engine queues issue strictly in program order, so by the time an
    # instruction issues, every wait of every EARLIER instruction on its
    # engine queue has been satisfied — those waits are free facts for the
    # implication closure (the Tile sem assigner itself relies on exactly
    # this order when it omits duplicate same-queue waits).
    prior_waits = [[] for _ in insts]
    eng_acc = {}
    for idx, ins in enumerate(insts):
        eng = ins.engine
        acc = eng_acc.setdefault(eng, [])
        prior_waits[idx] = list(acc)
        acc.extend(inst_waits[idx])

    n_stripped = 0
    for idx, ins in enumerate(insts):
        si = ins.sync_info
        if si is None or len(si.on_wait) <= 1:
            continue
        kept = list(si.on_wait)

        def key(w):
            return (w.id, w.wait_value, w.wait_mode == "sem-ge-imm" and w.wait_reg is None)

        progress = True
        while len(kept) >= 1 and progress:
            progress = False
            for w in list(kept):
                sid, v, clean = key(w)
                if not clean or sid in poisoned:
                    continue
                others = [key(k) for k in kept if k is not w] + prior_waits[idx]
                if facts_from(others).get(sid, 0) >= v:
                    kept.remove(w)
                    n_stripped += 1
                    progress = True
                    break
        limit = max_waits.get(type(ins).__name__)
        if limit is not None and len(kept) > limit:
            raise RuntimeError(
                f"{type(ins).__name__} {ins.name} still has {len(kept)} waits: "
                f"{[(w.ant_name, w.wait_value) for w in kept]}"
            )
        if len(kept) != len(si.on_wait):
            ins.sync_info = mybir.SyncInfo(on_wait=kept, on_update=list(si.on_update))

    # Second sweep: drop vacuous same-engine waits on the DVE. The DVE
    # retires strictly in order (per-op DRAIN), so a wait on the DVE's own
    # completion sem whose target value is reached by an earlier DVE
    # instruction in the stream is satisfied by construction.
    dve = mybir.EngineType.DVE
    stream_pos = {}
    pos = 0
    for idx, ins in enumerate(insts):
        if ins.engine == dve:
            stream_pos[idx] = pos
            pos += 1
    upd_engine_ok = {}  # sem_id -> True if all updaters are DVE non-DMA instrs
    for sid, ups in updaters.items():
        upd_engine_ok[sid] = all(
            insts[i].engine == dve
            and not isinstance(insts[i], (mybir.InstDMACopy, mybir.InstCollectiveCompute))
            for _, i in ups
        )
    for idx, ins in enumerate(insts):
        if ins.engine != dve:
            continue
        si = ins.sync_info
        if si is None or not si.on_wait:
            continue
        kept = []
        for w in si.on_wait:
            if (
                w.wait_mode == "sem-ge-imm"
                and w.wait_reg is None
                and w.id not in poisoned
                and upd_engine_ok.get(w.id)
            ):
                ups = updaters.get(w.id, [])
                first = next((i for cv, i in ups if cv >= w.wait_value), None)
                if first is not None and stream_pos.get(first, 1 << 60) < stream_pos[idx]:
                    n_stripped += 1
                    continue
            kept.append(w)
        if len(kept) != len(si.on_wait):
            ins.sync_info = mybir.SyncInfo(on_wait=kept, on_update=list(si.on_update))
    return n_stripped


def build_nc_scan(n_slabs: int = SLABS, bufs: int = 3, chunks: int = 2,
                  first_splits: int = 2, tail_groups: int = 2,
                  witness_lag: int = 2, strip: bool = True):
    """W-major hardware-scan kernel. Each slab arrives transposed to
    [w, h] (h contiguous), so one tensor_tensor_scan per chunk computes
    the per-column cummax: the mask input (-3e38 at h==0, else 0) resets
    the fp32 scan state at every column start, which also makes chunks
    and tiles fully independent — no spare columns, no cross-chunk state.
    Loads ride the SP HWDGE ring, stores the ACT ring. Stores need their
    lane-reuse waits strippable: each store gets a 1-elem DVE witness
    copy `witness_lag` chunks later (see module docstring)."""
    CHF = FREE // chunks
    assert CHF % H == 0 and n_slabs % P == 0
    n_tiles = n_slabs // P
    assert bufs < n_tiles, "slot-reuse WAR chains require bufs < n_tiles"

    nc = bass.Bass("TRN2", target_bir_lowering=False, debug=False)
    x = nc.dram_tensor("x", [n_slabs, FREE], DT, kind="ExternalInput").ap()
    out = nc.dram_tensor("out", [n_slabs, FREE], DT, kind="ExternalOutput").ap()

    pending = []               # stored chunk APs awaiting a witness copy
    with tile.TileContext(nc) as tc:
        with tc.tile_pool(name="mask", bufs=1) as mpool, \
                tc.tile_pool(name="work", bufs=bufs) as pool:
            mask = mpool.tile([P, CHF], DT)
            nc.vector.memset(mask, 0.0)
            mv = mask.rearrange("p (c h) -> p c h", h=H)
            nc.vector.memset(mv[:, :, 0:1], -3.0e38)
            for t in range(n_tiles):
                tl = pool.tile([P, FREE], DT)
                xrow = x[t * P:(t + 1) * P, :]
                orow = out[t * P:(t + 1) * P, :]
                for c in range(chunks):
                    c0, c1 = c * CHF, (c + 1) * CHF
                    ch = tl[:, c0:c1]
                    first_chunk = t == 0 and c == 0
                    last_chunk = t == n_tiles - 1 and c == chunks - 1
                    if last_chunk:
                        # all outstanding stores must be witnessed before
                        # the tail stores issue (their lane-reuse preds)
                        for pch in pending:
                            nc.vector.tensor_copy(
                                pch[0:1, CHF - 1:CHF], pch[0:1, CHF - 1:CHF])
                        pending.clear()
                    # pieces: split the first chunk so the DVE starts on
                    # piece 0 while piece 1 loads; split the tail chunk so
                    # the final store is small
                    pieces = (first_splits if first_chunk
                              else tail_groups if last_chunk else 1)
                    pp = CHF // pieces
                    assert pp % H == 0
                    for pc in range(pieces):
                        s0, s1 = pc * pp, (pc + 1) * pp
                        nc.sync.dma_start(
                            ch[:, s0:s1], xrow[:, c0 + s0:c0 + s1])
                        nc.vector.tensor_tensor_scan(
                            ch[:, s0:s1], mask[:, 0:pp], ch[:, s0:s1],
                            0.0, mybir.AluOpType.add, mybir.AluOpType.max)
                        if last_chunk:
                            nc.scalar.dma_start(
                                orow[:, c0 + s0:c0 + s1], ch[:, s0:s1])
                    if not last_chunk:
                        nc.scalar.dma_start(orow[:, c0:c1], ch)
                        pending.append(ch)
                        if len(pending) > witness_lag:
                            pch = pending.pop(0)
                            nc.vector.tensor_copy(
                                pch[0:1, CHF - 1:CHF], pch[0:1, CHF - 1:CHF])
                    else:
                        # joiner: fold the final store's completion into the
                        # DVE stream so the kernel drain needs one ring wait
                        nc.vector.tensor_copy(
                            ch[0:1, CHF - 1:CHF], ch[0:1, CHF - 1:CHF])

    if strip:
        _strip_instruction_waits(nc)
    return nc


def build_nc(n_slabs: int = SLABS, bufs: int = 4, blocks: int = 16, halves: int = 1,
             first_splits: int = 2, dma_splits: int = 1, witness_lag: int = 4,
             load_splits: int = 1, xpad: int = 0, opad: int = 0,
             strip: bool = True):
    """h-major two-level blocked scan, bf16, pure DVE compute.

    halves: split each tile's load/compute/store into this many h-chunks.
    Each SBUF tile carries a W-wide "spare" column ahead of the data
    holding the previous tile's running max slice, so every block's
    prev-prefix slice sits exactly W elements before the block start —
    step2 and step3 use one uniform AP family and step3 collapses to one
    stride-0-broadcast op per chunk.

    Loads ride the SP HWDGE ring; chunk stores ride SWDGE (own DMASW sem
    lanes — loads keep all 8 DMAHW lanes to themselves). Each is issued
    as dma_splits sub-DMAs: 1 MiB transfers give the inter-core HBM
    arbiter finer interleave boundaries, which evens out the per-run
    "loser core" whose streams starve while its stack partner bursts.
    The tail stores quarter-granular entirely on the otherwise-idle ACT
    HWDGE ring — on a starved core the SWDGE ring is backlogged exactly
    then, and the tail must not queue behind it.

    bufs=4 = n_tiles holds the ENTIRE 16 MiB input shard in SBUF
    (130 KiB/partition): loads free-run at full HBM rate from t=0 with no
    WAR coupling to stores/compute, which rides out the multi-us DMA
    outages observed when this core's HBM-stack partner bursts. Without
    WAR chains the stripper cannot witness store completion, so each
    chunk store gets a 1-elem DVE "witness" copy witness_lag chunks later
    (far enough that the store has already completed — a shorter lag
    stalls the DVE, measured -6us/core at lag 2) that folds its DMASW
    lane-sem into the DVE tick stream, keeping every DMA at one wait.

    first_splits splits the first chunk's load so the DVE starts ~3us
    sooner."""
    B = blocks
    S = H // B
    assert n_slabs % P == 0
    assert B % halves == 0
    n_tiles = n_slabs // P
    BH = B // halves           # blocks per chunk
    CHF = FREE // halves       # free elems per chunk
    BW = S * W                 # elements per block

    nc = bass.Bass("TRN2", target_bir_lowering=False, debug=False)
    # xpad/opad: extra elements of DRAM row pitch — shifts each row's HBM
    # channel phase to decorrelate the per-engine address streams from the
    # stack partner's (straggler-engine mitigation experiment)
    xoff = XOFF if xpad else 0
    ooff = OOFF if opad else 0
    x = nc.dram_tensor("x", [n_slabs, FREE + xpad], DT,
                       kind="ExternalInput").ap()[:, xoff:xoff + FREE]
    out = nc.dram_tensor("out", [n_slabs, FREE + opad], DT,
                         kind="ExternalOutput").ap()[:, ooff:ooff + FREE]

    # tile layout: [spare0 | chunk0 | spare1 | chunk1 | ...] — each chunk's
    # spare (W elems) holds the running-max slice entering that chunk, so
    # every block's prev-prefix slice sits exactly W elems before the block
    TW = halves * W + FREE
    pending = []               # stored chunk APs awaiting a witness copy
    with tile.TileContext(nc) as tc:
        with tc.tile_pool(name="work", bufs=bufs) as pool:
            for t in range(n_tiles):
                tl = pool.tile([P, TW], DT)
                # each tile is an independent set of slabs; block 0 has no
                # predecessor, so its step2 link and step3 term (max with
                # the -inf spare) are numeric no-ops and are skipped —
                # which also makes the spare memset dead
                xrow = x[t * P:(t + 1) * P, :]
                orow = out[t * P:(t + 1) * P, :]
                for h in range(halves):
                    base = h * (W + CHF)
                    c0, c1 = h * CHF, (h + 1) * CHF
                    ch = tl[:, base + W:base + W + CHF]
                    v = ch.rearrange("p (b s w) -> p b s w", b=BH, s=S, w=W)
                    prevlast = tl[:, base:base + CHF].rearrange(
                        "p (b s w) -> p b s w", b=BH, s=S, w=W)[:, :, 0, :]
                    first_chunk = t == 0 and h == 0
                    last_chunk = t == n_tiles - 1 and h == halves - 1

                    if not last_chunk:
                        # loads (SP HWDGE ring) + step1 (in-block scan),
                        # piecewise for the first chunk so the DVE starts
                        # on piece 0 while piece 1 loads. Loads must NOT be
                        # spread across both HWDGE rings: the SDMA engines
                        # round-robin across queues, and 2 load queues vs 1
                        # store queue starves the store stream (measured
                        # +6us mean/core).
                        done_segments = False
                        if halves == 1 and t < 3 and first_splits == 2:
                            # tiles 0-1, fully per-segment: load/step1/
                            # step2/step3/store of each 2 MiB segment runs
                            # while the next segment's load is in flight.
                            # Early tiles have no prefetch cushion yet, so
                            # whole-tile (4 MiB) step1 granularity stalls
                            # the DVE ~10us when HBM bandwidth is tight
                            # (measured); later tiles run ahead of the DVE
                            # and keep the cheaper 7-fat-op form.
                            seg = BH // 2
                            for hh in range(2):
                                sb0, sb1 = hh * seg, (hh + 1) * seg
                                if t == 0 and hh == 0:
                                    # ladder the very first load (512K,
                                    # 512K, 1M): the DVE's first step1
                                    # starts on 512 KiB instead of 2 MiB
                                    # (~4us earlier for ~1us of extra op
                                    # overhead, paid once)
                                    ladder = [(sb0, sb0 + 2),
                                              (sb0 + 2, sb0 + 4),
                                              (sb0 + 4, sb1)]
                                elif load_splits > 1:
                                    lsp = seg // load_splits
                                    ladder = [(sb0 + i * lsp,
                                               sb0 + (i + 1) * lsp)
                                              for i in range(load_splits)]
                                else:
                                    ladder = [(sb0, sb1)]
                                for lb0, lb1 in ladder:
                                    nc.sync.dma_start(
                                        ch[:, lb0 * BW:lb1 * BW],
                                        xrow[:, c0 + lb0 * BW:c0 + lb1 * BW])
                                    for j in range(1, S):
                                        nc.vector.tensor_max(
                                            v[:, lb0:lb1, j, :],
                                            v[:, lb0:lb1, j, :],
                                            v[:, lb0:lb1, j - 1, :])
                                a0 = max(sb0, 1)
                                for b in range(a0, sb1):
                                    nc.vector.tensor_max(
                                        v[:, b, S - 1, :], v[:, b, S - 1, :],
                                        prevlast[:, b, :])
                                pb = prevlast[:, a0:sb1, :].unsqueeze(2) \
                                    .broadcast_to([P, sb1 - a0, S - 1, W])
                                nc.vector.tensor_max(
                                    v[:, a0:sb1, 0:S - 1, :],
                                    v[:, a0:sb1, 0:S - 1, :], pb)
                                ssp = seg // dma_splits
                                for sp in range(dma_splits):
                                    qb0 = sb0 + sp * ssp
                                    qb1 = sb0 + (sp + 1) * ssp
                                    nc.scalar.dma_start(
                                        orow[:, c0 + qb0 * BW:c0 + qb1 * BW],
                                        ch[:, qb0 * BW:qb1 * BW])
                                    pending.append(
                                        ch[0:1, qb1 * BW - 1:qb1 * BW])
                            done_segments = True
                        elif first_chunk:
                            pieces = first_splits
                            bpp = BH // pieces
                            for pc in range(pieces):
                                b0, b1 = pc * bpp, (pc + 1) * bpp
                                nc.sync.dma_start(
                                    ch[:, b0 * BW:b1 * BW],
                                    xrow[:, c0 + b0 * BW:c0 + b1 * BW])
                                for j in range(1, S):
                                    nc.vector.tensor_max(
                                        v[:, b0:b1, j, :], v[:, b0:b1, j, :],
                                        v[:, b0:b1, j - 1, :])
                        else:
                            sw = CHF // dma_splits
                            for sp in range(dma_splits):
                                nc.sync.dma_start(
                                    ch[:, sp * sw:(sp + 1) * sw],
                                    xrow[:, c0 + sp * sw:c0 + (sp + 1) * sw])
                            for j in range(1, S):
                                nc.vector.tensor_max(
                                    v[:, :, j, :], v[:, :, j, :],
                                    v[:, :, j - 1, :])

                        # witness stores issued witness_lag chunks ago:
                        # 1-elem WAR copies folding their DMASW lane-sems
                        # into the DVE tick stream (see docstring). With
                        # halves=1 there are at most 8 SWDGE stores — no
                        # lane reuse — so this never fires.
                        while len(pending) >= witness_lag * dma_splits:
                            pch = pending.pop(0)
                            nc.vector.tensor_copy(pch, pch)

                        if halves == 1 and done_segments:
                            pass
                        elif halves == 1:
                            # hybrid cadence: tile-granular step1 (7 fat
                            # ops) but step2 chained in two 8-block
                            # segments, each followed by its half's step3
                            # and a 2 MiB store — the store stream keeps
                            # the half-tile rhythm that a single tile-end
                            # store would destroy (measured +8.7us drain)
                            seg = BH // 2
                            for hh in range(2):
                                sb0, sb1 = hh * seg, (hh + 1) * seg
                                a0 = max(sb0, 1)
                                for b in range(a0, sb1):
                                    nc.vector.tensor_max(
                                        v[:, b, S - 1, :], v[:, b, S - 1, :],
                                        prevlast[:, b, :])
                                pb = prevlast[:, a0:sb1, :].unsqueeze(2) \
                                    .broadcast_to([P, sb1 - a0, S - 1, W])
                                nc.vector.tensor_max(
                                    v[:, a0:sb1, 0:S - 1, :],
                                    v[:, a0:sb1, 0:S - 1, :], pb)
                                nc.scalar.dma_start(
                                    orow[:, c0 + sb0 * BW:c0 + sb1 * BW],
                                    ch[:, sb0 * BW:sb1 * BW])
                                pending.append(
                                    ch[0:1, sb1 * BW - 1:sb1 * BW])
                        else:
                            # step2: chain block-last slices through the spare
                            for b in range(1, BH):
                                nc.vector.tensor_max(
                                    v[:, b, S - 1, :], v[:, b, S - 1, :],
                                    prevlast[:, b, :])
                            # bridge the running max into the next chunk's
                            # spare
                            bridge = None
                            if h + 1 < halves:
                                bridge = nc.vector.tensor_copy(
                                    tl[:, base + W + CHF:base + 2 * W + CHF],
                                    v[:, BH - 1, S - 1, :])
                            # step3: one op — prev-block prefix into slices
                            # 0..S-2
                            pb = prevlast.unsqueeze(2).broadcast_to(
                                [P, BH, S - 1, W])
                            s3 = nc.vector.tensor_max(
                                v[:, :, 0:S - 1, :], v[:, :, 0:S - 1, :], pb)
                            if bridge is not None:
                                # keep the bridge's DVE tick below the
                                # store's wait target so the slot's readers
                                # stay within it
                                tile.add_dep_helper(
                                    s3.ins, bridge.ins, sync=False,
                                    reason="bridge copy before step3 so slot "
                                           "readers stay under the store's "
                                           "DVE wait")
                            sw = CHF // dma_splits
                            for sp in range(dma_splits):
                                st = nc.scalar.dma_start(
                                    orow[:, c0 + sp * sw:c0 + (sp + 1) * sw],
                                    ch[:, sp * sw:(sp + 1) * sw])
                                # force an explicit DVE wait: Tile would
                                # give later sub-stores only pool-queue
                                # order, leaving them an unstrippable
                                # {lane-reuse, sub-load-RAW} wait pair; a
                                # DVE>=step3 wait implies both
                                tile.add_dep_helper(
                                    st.ins, s3.ins, sync=True,
                                    reason="sub-store's single DVE wait "
                                           "implies its lane-reuse and "
                                           "sub-load waits")
                                pending.append(
                                    ch[0:1, (sp + 1) * sw - 1:(sp + 1) * sw])
                    else:
                        # tail: the pipeline runs dry here, so feed the DMA
                        # to the very end — half-granular loads+step1,
                        # quarter-granular step3+stores alternating SWDGE
                        # and the ACT ring so the drain runs on two rings
                        # in parallel
                        BQ = BH // 2
                        CQ = CHF // 2
                        for half in range(2):
                            hb0 = half * BQ
                            nc.sync.dma_start(
                                ch[:, half * CQ:(half + 1) * CQ],
                                xrow[:, c0 + half * CQ:c0 + (half + 1) * CQ])
                            for j in range(1, S):
                                nc.vector.tensor_max(
                                    v[:, hb0:hb0 + BQ, j, :],
                                    v[:, hb0:hb0 + BQ, j, :],
                                    v[:, hb0:hb0 + BQ, j - 1, :])
                            for b in range(max(hb0, 1), hb0 + BQ):
                                nc.vector.tensor_max(
                                    v[:, b, S - 1, :], v[:, b, S - 1, :],
                                    prevlast[:, b, :])
                            if half == 0:
                                qbnds = [hb0, hb0 + BQ // 2, hb0 + BQ]
                            else:
                                # last half: taper the pieces so the final
                                # store (the pipeline's drain) is 512 KiB
                                qbnds = [hb0, hb0 + BQ // 2,
                                         hb0 + 3 * BQ // 4, hb0 + BQ]
                            for q in range(len(qbnds) - 1):
                                qb0, qb1 = qbnds[q], qbnds[q + 1]
                                qc0 = qb0 * S * W
                                qc1 = qb1 * S * W
                                qa = max(qb0, 1)
                                pq = prevlast[:, qa:qb1, :].unsqueeze(2) \
                                    .broadcast_to([P, qb1 - qa, S - 1, W])
                                nc.vector.tensor_max(
                                    v[:, qa:qb1, 0:S - 1, :],
                                    v[:, qa:qb1, 0:S - 1, :], pq)
                                eng = nc.scalar
                                eng.dma_start(
                                    orow[:, c0 + qc0:c0 + qc1], ch[:, qc0:qc1])
                        # joiners: 1-elem WAR copies handing the DVE the two
                        # rings' final-store completion waits, so the kernel
                        # tail drain reduces to engine waits. Stores left in
                        # `pending` stay unwitnessed: ring-FIFO closure from
                        # the final SWDGE store covers them.
                        nc.vector.tensor_copy(
                            ch[0:1, 2 * CQ // 2:2 * CQ // 2 + 1],
                            ch[0:1, 2 * CQ // 2:2 * CQ // 2 + 1])
                        nc.vector.tensor_copy(
                            ch[0:1, CHF - 1:CHF], ch[0:1, CHF - 1:CHF])

    if strip:
        # validate strippability even when returning the unstripped module
        # (CoreSim's race detector doesn't model same-engine in-order
        # retirement, so sim runs pass strip=False)
        _strip_instruction_waits(nc)
    return nc


def _get_nc():
    key = "default"
    if key not in _NC_CACHE:
        # h-major blocked kernel only: tensor_tensor_scan measured 2.1
        # cyc/elem on HW (vs 0.5 for packed bf16 TT) AND miscomputed, so
        # the scan path is dead. The Tile scheduler is not perfectly
        # deterministic across processes; if a schedule ever leaves a DMA
        # with >1 sync wait the stripper raises. Retry, then fall back to
        # coarser structures whose stripping is trivially easy.
        nc = None
        layout = "hmajor"
        for attempt in range(3):
            try:
                nc = build_nc(xpad=XPAD, opad=OPAD)
                break
            except RuntimeError:
                continue
        if nc is None:
            for kwargs in (
                dict(first_splits=1),
                dict(first_splits=1, bufs=3),
                dict(first_splits=1, halves=1, bufs=3),
            ):
                try:
                    nc = build_nc(xpad=XPAD, opad=OPAD, **kwargs)
                    break
                except RuntimeError:
                    continue
        assert nc is not None, "all kernel builds failed wait-stripping"
        _NC_CACHE[key] = (nc, layout)
    return _NC_CACHE[key]


def _shard(x: np.ndarray, layout: str):
    per = BATCH // N_CORES
    xb = x.astype(NP_DT)
    if layout == "wmajor":
        xb = xb.transpose(0, 1, 3, 2)  # [B, C, W, H] — h contiguous per col
    shards = []
    for i in range(N_CORES):
        s = np.ascontiguousarray(xb[i * per:(i + 1) * per]).reshape(SLABS, FREE)
        if XPAD:
            sp = np.zeros((SLABS, FREE + XPAD), dtype=NP_DT)
            sp[:, XOFF:XOFF + FREE] = s
            s = sp
        shards.append(s)
    return shards


def _unshard(outs, layout: str):
    per = BATCH // N_CORES
    outs = [o[:, OOFF:OOFF + FREE] if OPAD else o for o in outs]
    if layout == "wmajor":
        shards = [o.reshape(per, CH, W, H).transpose(0, 1, 3, 2) for o in outs]
    else:
        shards = [o.reshape(per, CH, H, W) for o in outs]
    return np.concatenate([s.astype(np.float32) for s in shards], axis=0)


def run(x: np.ndarray, trace: bool = False, **kwargs):
    """Run on hardware; returns (full_output, BassKernelResults)."""
    x = np.asarray(x, dtype=np.float32)
    assert x.shape == (BATCH, CH, H, W), x.shape
    nc, layout = _get_nc()
    in_maps = [{"x": s} for s in _shard(x, layout)]
    res = run_bass_kernel_spmd(
        nc, in_maps, core_ids=list(range(N_CORES)), trace=trace, **kwargs
    )
    out = _unshard([res.results[i]["out"] for i in range(N_CORES)], layout)
    return out, res


def kernel(x) -> np.ndarray:
    out, _ = run(np.asarray(x), trace=False)
    return out



# revision 24
# speedup vs baseline: 1.0417x; 1.0037x over previous
"""BottomPool (cummax along H) for (16, 256, 128, 128) f32 on 8 TRN2 NeuronCores.

The device pipeline runs in bf16: max() selects values and introduces no
arithmetic rounding, so the only error is the host-side f32->bf16 input
cast (<= 2^-9 relative, ~10x under the 2e-2 gate) while HBM traffic —
the binding roofline — halves to 16+16 MiB per core.

Sharding: data-parallel over batch — 2 batches per core. Each core's
shard is viewed as [512 slabs, H*W] where a slab is one (b, c) image.
Partition dim = slab (128 slabs per SBUF tile).

Kernel (build_nc): h-major two-level blocked scan (step1 in-block /
step2 block chain / step3 prefix distribute) on the DVE, ~1.9 packed
bf16 passes (~73us/core busy) overlapped with full-shard SBUF prefetch.
Loads ride the SP HWDGE ring, stores the ACT HWDGE ring (SWDGE retired:
the loser-core straggler engine is NOT descriptor-ring contention — it
is HBM-pair arbitration; all-HWDGE is equal-or-better and simpler).
x/out DRAM rows are pitch-padded + column-offset (XPAD/XOFF/OPAD/OOFF)
to shift per-engine HBM channel phase: measured to clear the DMA_15-
type straggler on cores 0/6 (max-core unchanged ~108us, mean -3us).
(build_nc_scan, kept for reference, mapped the cummax onto the
``tensor_tensor_scan`` ISA op in a host-transposed w-major layout — one
DVE op per chunk — but on hardware the scan measured 2.1 cyc/elem vs
the 0.5 of packed TT ops AND miscomputed, so it is dead code.)

walrus codegen accepts only a small number of sync waits per instruction
(one for DMA pseudo-instructions), but Tile's sem assigner is not
transitively minimal and can attach more. strip_implied_waits() removes
waits that are provably implied: a wait is redundant when the kept waits'
completion closure (instructions that must have completed, including
same-HWDGE-ring FIFO predecessors of completed DMAs) already forces the
waited semaphore to the required value. Two structural invariants keep
every DMA at one wait: (a) bufs < n_tiles, so slot-reuse WAR chains let
the stripper witness store completion, and (b) every store gets a 1-elem
DVE "witness" copy a couple of chunks later that folds its DMAHW
lane-sem completion into the DVE tick stream.
"""

import ml_dtypes
import numpy as np

from concourse import bass, mybir, tile
from concourse.bass_utils import run_bass_kernel_spmd

N_CORES = 8
BATCH, CH, H, W = 16, 256, 128, 128
FREE = H * W
P = 128                      # slabs per tile = SBUF partitions
SLABS = (BATCH // N_CORES) * CH  # 512 slabs per core
DT = mybir.dt.bfloat16
NP_DT = ml_dtypes.bfloat16
XPAD = 64                     # DRAM row-pitch pad (elems) for x, see build_nc
XOFF = 0                      # data column offset inside the padded row
OPAD = 64                     # same for out
OOFF = 0

_NC_CACHE = {}


def _strip_instruction_waits(nc, max_waits={"InstDMACopy": 1, "InstDrain": 1}):
    insts = []
    for f in nc.m.functions:
        for b in f.blocks:
            insts.extend(b.instructions)

    # Monotone-sem updater table: sem_id -> [(cum_value_after, inst_idx)].
    # Sems touched by non-monotone updates are excluded entirely.
    poisoned = set()
    cum = {}
    updaters = {}
    inst_updates = [[] for _ in insts]  # idx -> [(sem_id, cum_after)]
    for idx, ins in enumerate(insts):
        si = ins.sync_info
        if si is None:
            continue
        for u in si.on_update:
            if u.update_mode == "sem-add-imm" and u.update_reg is None:
                val = u.update_value
            elif u.update_mode == "sem-inc":
                val = 1
            else:
                poisoned.add(u.id)
                continue
            cum[u.id] = cum.get(u.id, 0) + val
            updaters.setdefault(u.id, []).append((cum[u.id], idx))
            inst_updates[idx].append((u.id, cum[u.id]))

    # Same-HWDGE-ring FIFO order: DMAs issued on one ring complete in
    # program order, so a later DMA's completion implies all earlier ones.
    ring_pos = {}   # inst_idx -> (queue, position)
    ring_members = {}  # queue -> [inst_idx in order]
    for idx, ins in enumerate(insts):
        if isinstance(ins, mybir.InstDMACopy):
            q = ins.queue
            ring_members.setdefault(q, []).append(idx)
            ring_pos[idx] = (q, len(ring_members[q]) - 1)

    inst_waits = []
    for ins in insts:
        si = ins.sync_info
        ws = []
        if si is not None:
            for w in si.on_wait:
                if w.wait_mode == "sem-ge-imm" and w.wait_reg is None:
                    ws.append((w.id, w.wait_value, True))
                else:
                    ws.append((w.id, w.wait_value, False))
        inst_waits.append(ws)

    def facts_from(seed_waits):
        """Fixpoint: semaphore lower bounds guaranteed once seed_waits hold."""
        facts = {}
        for sid, v, clean in seed_waits:
            if clean and sid not in poisoned:
                facts[sid] = max(facts.get(sid, 0), v)
        completed = set()
        changed = True
        while changed:
            changed = False
            for sid, v in list(facts.items()):
                for cval, idx in updaters.get(sid, []):
                    if cval > v:
                        break
                    if idx not in completed:
                        completed.add(idx)
                        changed = True
            for idx in list(completed):
                rp = ring_pos.get(idx)
                if rp is not None:
                    q, pos = rp
                    for pidx in ring_members[q][:pos]:
                        if pidx not in completed:
                            completed.add(pidx)
                            changed = True
            for idx in list(completed):
                for sid, v, clean in inst_waits[idx]:
                    if clean and sid not in poisoned and facts.get(sid, 0) < v:
                        facts[sid] = v
                        changed = True
                for sid, cval in inst_updates[idx]:
                    if sid not in poisoned and facts.get(sid, 0) < cval:
                        facts[sid] = cval
                        changed = True
        return facts

    # Engine queues issue strictly in program order, so by the time an
    # instruction issues, every wait of every EARLIER instruction on its
    # engine queue has been satisfied — those waits are free facts for the
    # implication closure (the Tile sem assigner itself relies on exactly
    # this order when it omits duplicate same-queue waits).
    prior_waits = [[] for _ in insts]
    eng_acc = {}
    for idx, ins in enumerate(insts):
        eng = ins.engine
        acc = eng_acc.setdefault(eng, [])
        prior_waits[idx] = list(acc)
        acc.extend(inst_waits[idx])

    n_stripped = 0
    for idx, ins in enumerate(insts):
        si = ins.sync_info
        if si is None or len(si.on_wait) <= 1:
            continue
        kept = list(si.on_wait)

        def key(w):
            return (w.id, w.wait_value, w.wait_mode == "sem-ge-imm" and w.wait_reg is None)

        progress = True
        while len(kept) >= 1 and progress:
            progress = False
            for w in list(kept):
                sid, v, clean = key(w)
                if not clean or sid in poisoned:
                    continue
                others = [key(k) for k in kept if k is not w] + prior_waits[idx]
                if facts_from(others).get(sid, 0) >= v:
                    kept.remove(w)
                    n_stripped += 1
                    progress = True
                    break
        limit = max_waits.get(type(ins).__name__)
        if limit is not None and len(kept) > limit:
            raise RuntimeError(
                f"{type(ins).__name__} {ins.name} still has {len(kept)} waits: "
                f"{[(w.ant_name, w.wait_value) for w in kept]}"
            )
        if len(kept) != len(si.on_wait):
            ins.sync_info = mybir.SyncInfo(on_wait=kept, on_update=list(si.on_update))

    # Second sweep: drop vacuous same-engine waits on the DVE. The DVE
    # retires strictly in order (per-op DRAIN), so a wait on the DVE's own
    # completion sem whose target value is reached by an earlier DVE
    # instruction in the stream is satisfied by construction.
    dve = mybir.EngineType.DVE
    stream_pos = {}
    pos = 0
    for idx, ins in enumerate(insts):
        if ins.engine == dve:
            stream_pos[idx] = pos
            pos += 1
    upd_engine_ok = {}  # sem_id -> True if all updaters are DVE non-DMA instrs
    for sid, ups in updaters.items():
        upd_engine_ok[sid] = all(
            insts[i].engine == dve
            and not isinstance(insts[i], (mybir.InstDMACopy, mybir.InstCollectiveCompute))
            for _, i in ups
        )
    for idx, ins in enumerate(insts):
        if ins.engine != dve:
            continue
        si = ins.sync_info
        if si is None or not si.on_wait:
            continue
        kept = []
        for w in si.on_wait:
            if (
                w.wait_mode == "sem-ge-imm"
                and w.wait_reg is None
                and w.id not in poisoned
                and upd_engine_ok.get(w.id)
            ):
                ups = updaters.get(w.id, [])
                first = next((i for cv, i in ups if cv >= w.wait_value), None)
                if first is not None and stream_pos.get(first, 1 << 60) < stream_pos[idx]:
                    n_stripped += 1
                    continue
            kept.append(w)
        if len(kept) != len(si.on_wait):
            ins.sync_info = mybir.SyncInfo(on_wait=kept, on_update=list(si.on_update))
    return n_stripped


def build_nc_scan(n_slabs: int = SLABS, bufs: int = 3, chunks: int = 2,
                  first_splits: int = 2, tail_groups: int = 2,
                  witness_lag: int = 2, strip: bool = True):
    """W-major hardware-scan kernel. Each slab arrives transposed to
    [w, h] (h contiguous), so one tensor_tensor_scan per chunk computes
    the per-column cummax: the mask input (-3e38 at h==0, else 0) resets
    the fp32 scan state at every column start, which also makes chunks
    and tiles fully independent — no spare columns, no cross-chunk state.
    Loads ride the SP HWDGE ring, stores the ACT ring. Stores need their
    lane-reuse waits strippable: each store gets a 1-elem DVE witness
    copy `witness_lag` chunks later (see module docstring)."""
    CHF = FREE // chunks
    assert CHF % H == 0 and n_slabs % P == 0
    n_tiles = n_slabs // P
    assert bufs < n_tiles, "slot-reuse WAR chains require bufs < n_tiles"

    nc = bass.Bass("TRN2", target_bir_lowering=False, debug=False)
    x = nc.dram_tensor("x", [n_slabs, FREE], DT, kind="ExternalInput").ap()
    out = nc.dram_tensor("out", [n_slabs, FREE], DT, kind="ExternalOutput").ap()

    pending = []               # stored chunk APs awaiting a witness copy
    with tile.TileContext(nc) as tc:
        with tc.tile_pool(name="mask", bufs=1) as mpool, \
                tc.tile_pool(name="work", bufs=bufs) as pool:
            mask = mpool.tile([P, CHF], DT)
            nc.vector.memset(mask, 0.0)
            mv = mask.rearrange("p (c h) -> p c h", h=H)
            nc.vector.memset(mv[:, :, 0:1], -3.0e38)
            for t in range(n_tiles):
                tl = pool.tile([P, FREE], DT)
                xrow = x[t * P:(t + 1) * P, :]
                orow = out[t * P:(t + 1) * P, :]
                for c in range(chunks):
                    c0, c1 = c * CHF, (c + 1) * CHF
                    ch = tl[:, c0:c1]
                    first_chunk = t == 0 and c == 0
                    last_chunk = t == n_tiles - 1 and c == chunks - 1
                    if last_chunk:
                        # all outstanding stores must be witnessed before
                        # the tail stores issue (their lane-reuse preds)
                        for pch in pending:
                            nc.vector.tensor_copy(
                                pch[0:1, CHF - 1:CHF], pch[0:1, CHF - 1:CHF])
                        pending.clear()
                    # pieces: split the first chunk so the DVE starts on
                    # piece 0 while piece 1 loads; split the tail chunk so
                    # the final store is small
                    pieces = (first_splits if first_chunk
                              else tail_groups if last_chunk else 1)
                    pp = CHF // pieces
                    assert pp % H == 0
                    for pc in range(pieces):
                        s0, s1 = pc * pp, (pc + 1) * pp
                        nc.sync.dma_start(
                            ch[:, s0:s1], xrow[:, c0 + s0:c0 + s1])
                        nc.vector.tensor_tensor_scan(
                            ch[:, s0:s1], mask[:, 0:pp], ch[:, s0:s1],
                            0.0, mybir.AluOpType.add, mybir.AluOpType.max)
                        if last_chunk:
                            nc.scalar.dma_start(
                                orow[:, c0 + s0:c0 + s1], ch[:, s0:s1])
                    if not last_chunk:
                        nc.scalar.dma_start(orow[:, c0:c1], ch)
                        pending.append(ch)
                        if len(pending) > witness_lag:
                            pch = pending.pop(0)
                            nc.vector.tensor_copy(
                                pch[0:1, CHF - 1:CHF], pch[0:1, CHF - 1:CHF])
                    else:
                        # joiner: fold the final store's completion into the
                        # DVE stream so the kernel drain needs one ring wait
                        nc.vector.tensor_copy(
                            ch[0:1, CHF - 1:CHF], ch[0:1, CHF - 1:CHF])

    if strip:
        _strip_instruction_waits(nc)
    return nc


def build_nc(n_slabs: int = SLABS, bufs: int = 4, blocks: int = 16, halves: int = 1,
             first_splits: int = 2, dma_splits: int = 1, witness_lag: int = 4,
             load_splits: int = 1, xpad: int = 0, opad: int = 0,
             strip: bool = True):
    """h-major two-level blocked scan, bf16, pure DVE compute.

    halves: split each tile's load/compute/store into this many h-chunks.
    Each SBUF tile carries a W-wide "spare" column ahead of the data
    holding the previous tile's running max slice, so every block's
    prev-prefix slice sits exactly W elements before the block start —
    step2 and step3 use one uniform AP family and step3 collapses to one
    stride-0-broadcast op per chunk.

    Loads ride the SP HWDGE ring; chunk stores ride SWDGE (own DMASW sem
    lanes — loads keep all 8 DMAHW lanes to themselves). Each is issued
    as dma_splits sub-DMAs: 1 MiB transfers give the inter-core HBM
    arbiter finer interleave boundaries, which evens out the per-run
    "loser core" whose streams starve while its stack partner bursts.
    The tail stores quarter-granular entirely on the otherwise-idle ACT
    HWDGE ring — on a starved core the SWDGE ring is backlogged exactly
    then, and the tail must not queue behind it.

    bufs=4 = n_tiles holds the ENTIRE 16 MiB input shard in SBUF
    (130 KiB/partition): loads free-run at full HBM rate from t=0 with no
    WAR coupling to stores/compute, which rides out the multi-us DMA
    outages observed when this core's HBM-stack partner bursts. Without
    WAR chains the stripper cannot witness store completion, so each
    chunk store gets a 1-elem DVE "witness" copy witness_lag chunks later
    (far enough that the store has already completed — a shorter lag
    stalls the DVE, measured -6us/core at lag 2) that folds its DMASW
    lane-sem into the DVE tick stream, keeping every DMA at one wait.

    first_splits splits the first chunk's load so the DVE starts ~3us
    sooner."""
    B = blocks
    S = H // B
    assert n_slabs % P == 0
    assert B % halves == 0
    n_tiles = n_slabs // P
    BH = B // halves           # blocks per chunk
    CHF = FREE // halves       # free elems per chunk
    BW = S * W                 # elements per block

    nc = bass.Bass("TRN2", target_bir_lowering=False, debug=False)
    # xpad/opad: extra elements of DRAM row pitch — shifts each row's HBM
    # channel phase to decorrelate the per-engine address streams from the
    # stack partner's (straggler-engine mitigation experiment)
    xoff = XOFF if xpad else 0
    ooff = OOFF if opad else 0
    x = nc.dram_tensor("x", [n_slabs, FREE + xpad], DT,
                       kind="ExternalInput").ap()[:, xoff:xoff + FREE]
    out = nc.dram_tensor("out", [n_slabs, FREE + opad], DT,
                         kind="ExternalOutput").ap()[:, ooff:ooff + FREE]

    # tile layout: [spare0 | chunk0 | spare1 | chunk1 | ...] — each chunk's
    # spare (W elems) holds the running-max slice entering that chunk, so
    # every block's prev-prefix slice sits exactly W elems before the block
    TW = halves * W + FREE
    # scratch output for the ACT-ring priming store (never read back)
    prime = nc.dram_tensor("prime", [1, 256], DT, kind="ExternalOutput").ap()
    pending = []               # stored chunk APs awaiting a witness copy
    with tile.TileContext(nc) as tc:
        with tc.tile_pool(name="prime", bufs=1) as ppool, \
                tc.tile_pool(name="work", bufs=bufs) as pool:
            # prime the ACT HWDGE ring at t~6us: its first transfer pays a
            # ~3.6us arm latency (measured trigger->first-data 22.1->26.5us,
            # vs 0.8us on the already-warm SP ring), which otherwise lands
            # on the critical first real store
            pr = ppool.tile([1, 256], DT)
            nc.vector.memset(pr, 0.0)
            nc.scalar.dma_start(prime[0:1, :], pr)
            # witness it via the pending queue (fires a few chunks in, long
            # after the 512B store completed) so lane reuse stays strippable
            pending.append(pr[0:1, 0:1])
            for t in range(n_tiles):
                tl = pool.tile([P, TW], DT)
                # each tile is an independent set of slabs; block 0 has no
                # predecessor, so its step2 link and step3 term (max with
                # the -inf spare) are numeric no-ops and are skipped —
                # which also makes the spare memset dead
                xrow = x[t * P:(t + 1) * P, :]
                orow = out[t * P:(t + 1) * P, :]
                for h in range(halves):
                    base = h * (W + CHF)
                    c0, c1 = h * CHF, (h + 1) * CHF
                    ch = tl[:, base + W:base + W + CHF]
                    v = ch.rearrange("p (b s w) -> p b s w", b=BH, s=S, w=W)
                    prevlast = tl[:, base:base + CHF].rearrange(
                        "p (b s w) -> p b s w", b=BH, s=S, w=W)[:, :, 0, :]
                    first_chunk = t == 0 and h == 0
                    last_chunk = t == n_tiles - 1 and h == halves - 1

                    if not last_chunk:
                        # loads (SP HWDGE ring) + step1 (in-block scan),
                        # piecewise for the first chunk so the DVE starts
                        # on piece 0 while piece 1 loads. Loads must NOT be
                        # spread across both HWDGE rings: the SDMA engines
                        # round-robin across queues, and 2 load queues vs 1
                        # store queue starves the store stream (measured
                        # +6us mean/core).
                        done_segments = False
                        if halves == 1 and t < 3 and first_splits == 2:
                            # tiles 0-1, fully per-segment: load/step1/
                            # step2/step3/store of each 2 MiB segment runs
                            # while the next segment's load is in flight.
                            # Early tiles have no prefetch cushion yet, so
                            # whole-tile (4 MiB) step1 granularity stalls
                            # the DVE ~10us when HBM bandwidth is tight
                            # (measured); later tiles run ahead of the DVE
                            # and keep the cheaper 7-fat-op form.
                            seg = BH // 2
                            for hh in range(2):
                                sb0, sb1 = hh * seg, (hh + 1) * seg
                                if t == 0 and hh == 0:
                                    # ladder the very first load (512K,
                                    # 512K, 1M): the DVE's first step1
                                    # starts on 512 KiB instead of 2 MiB
                                    # (~4us earlier for ~1us of extra op
                                    # overhead, paid once)
                                    ladder = [(sb0, sb0 + 2),
                                              (sb0 + 2, sb0 + 4),
                                              (sb0 + 4, sb1)]
                                elif load_splits > 1:
                                    lsp = seg // load_splits
                                    ladder = [(sb0 + i * lsp,
                                               sb0 + (i + 1) * lsp)
                                              for i in range(load_splits)]
                                else:
                                    ladder = [(sb0, sb1)]
                                for lb0, lb1 in ladder:
                                    nc.sync.dma_start(
                                        ch[:, lb0 * BW:lb1 * BW],
                                        xrow[:, c0 + lb0 * BW:c0 + lb1 * BW])
                                    for j in range(1, S):
                                        nc.vector.tensor_max(
                                            v[:, lb0:lb1, j, :],
                                            v[:, lb0:lb1, j, :],
                                            v[:, lb0:lb1, j - 1, :])
                                a0 = max(sb0, 1)
                                for b in range(a0, sb1):
                                    nc.vector.tensor_max(
                                        v[:, b, S - 1, :], v[:, b, S - 1, :],
                                        prevlast[:, b, :])
                                pb = prevlast[:, a0:sb1, :].unsqueeze(2) \
                                    .broadcast_to([P, sb1 - a0, S - 1, W])
                                nc.vector.tensor_max(
                                    v[:, a0:sb1, 0:S - 1, :],
                                    v[:, a0:sb1, 0:S - 1, :], pb)
                                ssp = seg // dma_splits
                                for sp in range(dma_splits):
                                    qb0 = sb0 + sp * ssp
                                    qb1 = sb0 + (sp + 1) * ssp
                                    nc.scalar.dma_start(
                                        orow[:, c0 + qb0 * BW:c0 + qb1 * BW],
                                        ch[:, qb0 * BW:qb1 * BW])
                                    pending.append(
                                        ch[0:1, qb1 * BW - 1:qb1 * BW])
                            done_segments = True
                        elif first_chunk:
                            pieces = first_splits
                            bpp = BH // pieces
                            for pc in range(pieces):
                                b0, b1 = pc * bpp, (pc + 1) * bpp
                                nc.sync.dma_start(
                                    ch[:, b0 * BW:b1 * BW],
                                    xrow[:, c0 + b0 * BW:c0 + b1 * BW])
                                for j in range(1, S):
                                    nc.vector.tensor_max(
                                        v[:, b0:b1, j, :], v[:, b0:b1, j, :],
                                        v[:, b0:b1, j - 1, :])
                        else:
                            sw = CHF // dma_splits
                            for sp in range(dma_splits):
                                nc.sync.dma_start(
                                    ch[:, sp * sw:(sp + 1) * sw],
                                    xrow[:, c0 + sp * sw:c0 + (sp + 1) * sw])
                            for j in range(1, S):
                                nc.vector.tensor_max(
                                    v[:, :, j, :], v[:, :, j, :],
                                    v[:, :, j - 1, :])

                        # witness stores issued witness_lag chunks ago:
                        # 1-elem WAR copies folding their DMASW lane-sems
                        # into the DVE tick stream (see docstring). With
                        # halves=1 there are at most 8 SWDGE stores — no
                        # lane reuse — so this never fires.
                        while len(pending) >= witness_lag * dma_splits:
                            pch = pending.pop(0)
                            nc.vector.tensor_copy(pch, pch)

                        if halves == 1 and done_segments:
                            pass
                        elif halves == 1:
                            # hybrid cadence: tile-granular step1 (7 fat
                            # ops) but step2 chained in two 8-block
                            # segments, each followed by its half's step3
                            # and a 2 MiB store — the store stream keeps
                            # the half-tile rhythm that a single tile-end
                            # store would destroy (measured +8.7us drain)
                            seg = BH // 2
                            for hh in range(2):
                                sb0, sb1 = hh * seg, (hh + 1) * seg
                                a0 = max(sb0, 1)
                                for b in range(a0, sb1):
                                    nc.vector.tensor_max(
                                        v[:, b, S - 1, :], v[:, b, S - 1, :],
                                        prevlast[:, b, :])
                                pb = prevlast[:, a0:sb1, :].unsqueeze(2) \
                                    .broadcast_to([P, sb1 - a0, S - 1, W])
                                nc.vector.tensor_max(
                                    v[:, a0:sb1, 0:S - 1, :],
                                    v[:, a0:sb1, 0:S - 1, :], pb)
                                nc.scalar.dma_start(
                                    orow[:, c0 + sb0 * BW:c0 + sb1 * BW],
                                    ch[:, sb0 * BW:sb1 * BW])
                                pending.append(
                                    ch[0:1, sb1 * BW - 1:sb1 * BW])
                        else:
                            # step2: chain block-last slices through the spare
                            for b in range(1, BH):
                                nc.vector.tensor_max(
                                    v[:, b, S - 1, :], v[:, b, S - 1, :],
                                    prevlast[:, b, :])
                            # bridge the running max into the next chunk's
                            # spare
                            bridge = None
                            if h + 1 < halves:
                                bridge = nc.vector.tensor_copy(
                                    tl[:, base + W + CHF:base + 2 * W + CHF],
                                    v[:, BH - 1, S - 1, :])
                            # step3: one op — prev-block prefix into slices
                            # 0..S-2
                            pb = prevlast.unsqueeze(2).broadcast_to(
                                [P, BH, S - 1, W])
                            s3 = nc.vector.tensor_max(
                                v[:, :, 0:S - 1, :], v[:, :, 0:S - 1, :], pb)
                            if bridge is not None:
                                # keep the bridge's DVE tick below the
                                # store's wait target so the slot's readers
                                # stay within it
                                tile.add_dep_helper(
                                    s3.ins, bridge.ins, sync=False,
                                    reason="bridge copy before step3 so slot "
                                           "readers stay under the store's "
                                           "DVE wait")
                            sw = CHF // dma_splits
                            for sp in range(dma_splits):
                                st = nc.scalar.dma_start(
                                    orow[:, c0 + sp * sw:c0 + (sp + 1) * sw],
                                    ch[:, sp * sw:(sp + 1) * sw])
                                # force an explicit DVE wait: Tile would
                                # give later sub-stores only pool-queue
                                # order, leaving them an unstrippable
                                # {lane-reuse, sub-load-RAW} wait pair; a
                                # DVE>=step3 wait implies both
                                tile.add_dep_helper(
                                    st.ins, s3.ins, sync=True,
                                    reason="sub-store's single DVE wait "
                                           "implies its lane-reuse and "
                                           "sub-load waits")
                                pending.append(
                                    ch[0:1, (sp + 1) * sw - 1:(sp + 1) * sw])
                    else:
                        # tail: the pipeline runs dry here, so feed the DMA
                        # to the very end — half-granular loads+step1,
                        # quarter-granular step3+stores alternating SWDGE
                        # and the ACT ring so the drain runs on two rings
                        # in parallel
                        BQ = BH // 2
                        CQ = CHF // 2
                        for half in range(2):
                            hb0 = half * BQ
                            nc.sync.dma_start(
                                ch[:, half * CQ:(half + 1) * CQ],
                                xrow[:, c0 + half * CQ:c0 + (half + 1) * CQ])
                            for j in range(1, S):
                                nc.vector.tensor_max(
                                    v[:, hb0:hb0 + BQ, j, :],
                                    v[:, hb0:hb0 + BQ, j, :],
                                    v[:, hb0:hb0 + BQ, j - 1, :])
                            for b in range(max(hb0, 1), hb0 + BQ):
                                nc.vector.tensor_max(
                                    v[:, b, S - 1, :], v[:, b, S - 1, :],
                                    prevlast[:, b, :])
                            if half == 0:
                                qbnds = [hb0, hb0 + BQ // 2, hb0 + BQ]
                            else:
                                # last half: taper the pieces so the final
                                # store (the pipeline's drain) is 512 KiB
                                qbnds = [hb0, hb0 + BQ // 2,
                                         hb0 + 3 * BQ // 4, hb0 + BQ]
                            for q in range(len(qbnds) - 1):
                                qb0, qb1 = qbnds[q], qbnds[q + 1]
                                qc0 = qb0 * S * W
                                qc1 = qb1 * S * W
                                qa = max(qb0, 1)
                                pq = prevlast[:, qa:qb1, :].unsqueeze(2) \
                                    .broadcast_to([P, qb1 - qa, S - 1, W])
                                nc.vector.tensor_max(
                                    v[:, qa:qb1, 0:S - 1, :],
                                    v[:, qa:qb1, 0:S - 1, :], pq)
                                eng = nc.scalar
                                eng.dma_start(
                                    orow[:, c0 + qc0:c0 + qc1], ch[:, qc0:qc1])
                        # (tail joiner copies removed: the final stores'
                        # completion is awaited directly by the kernel-end
                        # drains — saves one DVE hop (~0.6us) after the last
                        # store's data lands. Stores left in `pending` stay
                        # unwitnessed: ring-FIFO closure from the final
                        # ACT-ring store covers them.)

    if strip:
        # validate strippability even when returning the unstripped module
        # (CoreSim's race detector doesn't model same-engine in-order
        # retirement, so sim runs pass strip=False)
        _strip_instruction_waits(nc)
    return nc


def _get_nc():
    key = "default"
    if key not in _NC_CACHE:
        # h-major blocked kernel only: tensor_tensor_scan measured 2.1
        # cyc/elem on HW (vs 0.5 for packed bf16 TT) AND miscomputed, so
        # the scan path is dead. The Tile scheduler is not perfectly
        # deterministic across processes; if a schedule ever leaves a DMA
        # with >1 sync wait the stripper raises. Retry, then fall back to
        # coarser structures whose stripping is trivially easy.
        nc = None
        layout = "hmajor"
        for attempt in range(3):
            try:
                nc = build_nc(xpad=XPAD, opad=OPAD)
                break
            except RuntimeError:
                continue
        if nc is None:
            for kwargs in (
                dict(first_splits=1),
                dict(first_splits=1, bufs=3),
                dict(first_splits=1, halves=1, bufs=3),
            ):
                try:
                    nc = build_nc(xpad=XPAD, opad=OPAD, **kwargs)
                    break
                except RuntimeError:
                    continue
        assert nc is not None, "all kernel builds failed wait-stripping"
        _NC_CACHE[key] = (nc, layout)
    return _NC_CACHE[key]


def _shard(x: np.ndarray, layout: str):
    per = BATCH // N_CORES
    xb = x.astype(NP_DT)
    if layout == "wmajor":
        xb = xb.transpose(0, 1, 3, 2)  # [B, C, W, H] — h contiguous per col
    shards = []
    for i in range(N_CORES):
        s = np.ascontiguousarray(xb[i * per:(i + 1) * per]).reshape(SLABS, FREE)
        if XPAD:
            sp = np.zeros((SLABS, FREE + XPAD), dtype=NP_DT)
            sp[:, XOFF:XOFF + FREE] = s
            s = sp
        shards.append(s)
    return shards


def _unshard(outs, layout: str):
    per = BATCH // N_CORES
    outs = [o[:, OOFF:OOFF + FREE] if OPAD else o for o in outs]
    if layout == "wmajor":
        shards = [o.reshape(per, CH, W, H).transpose(0, 1, 3, 2) for o in outs]
    else:
        shards = [o.reshape(per, CH, H, W) for o in outs]
    return np.concatenate([s.astype(np.float32) for s in shards], axis=0)


def run(x: np.ndarray, trace: bool = False, **kwargs):
    """Run on hardware; returns (full_output, BassKernelResults)."""
    x = np.asarray(x, dtype=np.float32)
    assert x.shape == (BATCH, CH, H, W), x.shape
    nc, layout = _get_nc()
    in_maps = [{"x": s} for s in _shard(x, layout)]
    res = run_bass_kernel_spmd(
        nc, in_maps, core_ids=list(range(N_CORES)), trace=trace, **kwargs
    )
    out = _unshard([res.results[i]["out"] for i in range(N_CORES)], layout)
    return out, res


def kernel(x) -> np.ndarray:
    out, _ = run(np.asarray(x), trace=False)
    return out

